# revision 3
# baseline (speedup 1.0000x reference)
"""ResNet BasicBlock (conv3x3-BN-ReLU-conv3x3-BN-add-ReLU) on 8 TRN2 NeuronCores.

Data-parallel over batch (4 images per core). Convs are implicit GEMM on the
TensorEngine: 9 shifted-window bf16 matmuls accumulated per PSUM row-tile,
issued tap-major over groups of 4 tiles so one LDWEIGHTS serves 4 matmuls
(a post-legalize pass drops the redundant weight loads). Training-mode BN is
exact sync-BN: per-core (sum, sumsq) partials are transposed on the PE into a
[2,C] tile so the collective-input DMA is 2 contiguous lines instead of a
128-partition scatter (which cost ~8us of doorbell latency), AllGathered, and
transposed back for the per-channel coefficient math. A throwaway AllGather at
kernel start absorbs the cross-core collective-init barrier; a dummy Sqrt
preloads the ACT table set. The BN rsqrt runs as ACT Sqrt + one Newton step +
DVE reciprocal. conv2 output, the fused relu(scale*y + bias + x) epilogue and
the output DMA all run in bf16 (host upcasts), halving DVE/ACT/HBM cost of the
tail; output DMAs alternate between two queues.
"""

import functools
from contextlib import ExitStack

import ml_dtypes
import numpy as np

from concourse import bacc, bass, masks, mybir, tile
from concourse.bass_utils import run_bass_kernel_spmd

F32 = mybir.dt.float32
BF16 = mybir.dt.bfloat16
AF = mybir.ActivationFunctionType
ALU = mybir.AluOpType

N_CORES = 8
B, C, H, W = 32, 128, 56, 56
B_SH = B // N_CORES           # 4 images per core
HP, WP = H + 2, W + 2         # 58 (zero-padded)
ROWS = 8                      # output rows per conv tile
TPB = H // ROWS               # 7 tiles per image
NT = B_SH * TPB               # 28 tiles per conv per core
GRP = 4                       # conv tiles sharing one weight load per tap
N_GLOB = B * H * W            # BN sample count
EPS = 1e-5


def _dedupe_ldweights(nc):
    """Drop InstLdweights that reload the exact weights already resident in
    the PE array (tap-major matmul groups make runs of 4 identical loads).
    Only loads with no sync_info are dropped; any other PE instruction
    conservatively invalidates the tracked state."""
    PE = mybir.EngineType.PE
    for f in nc.m.functions:
        for bb in f.blocks:
            il = bb.instructions
            keep = []
            last_sig = None
            for inst in il:
                tn = type(inst).__name__
                if tn == "InstLdweights":
                    ap = inst.ins[0]
                    sig = (
                        str(ap),
                        str(inst.perf_mode),
                        str(inst.is_transpose),
                        str(inst.tile_position),
                    )
                    si = inst.sync_info
                    clean = si is None or (
                        len(si.on_wait) == 0 and len(si.on_update) == 0
                    )
                    if clean and sig == last_sig:
                        continue  # redundant reload of resident weights
                    last_sig = sig
                    keep.append(inst)
                    continue
                keep.append(inst)
                if getattr(inst, "engine", None) == PE:
                    if tn == "InstMatmult" and inst.ldweights is False and not inst.is_transpose:
                        pass  # consumes resident weights, does not change them
                    else:
                        last_sig = None
            if len(keep) != len(il):
                il.clear()
                il.extend(keep)


def _build():
    nc = bacc.Bacc(
        "TRN2",
        target_bir_lowering=False,
        debug=False,
        enable_asserts=False,
        num_devices=N_CORES,
    )

    xp_d = nc.dram_tensor("xp", [B_SH, C, HP, WP], BF16, kind="ExternalInput")
    w1_d = nc.dram_tensor("w1t", [C, 9 * C], BF16, kind="ExternalInput")
    w2_d = nc.dram_tensor("w2t", [C, 9 * C], BF16, kind="ExternalInput")
    g1_d = nc.dram_tensor("g1", [C, 1], F32, kind="ExternalInput")
    b1_d = nc.dram_tensor("b1", [C, 1], F32, kind="ExternalInput")
    g2_d = nc.dram_tensor("g2", [C, 1], F32, kind="ExternalInput")
    b2_d = nc.dram_tensor("b2", [C, 1], F32, kind="ExternalInput")
    out_d = nc.dram_tensor("out", [B_SH, C, H, W], BF16, kind="ExternalOutput")

    with tile.TileContext(nc) as tc, ExitStack() as ctx:
        const = ctx.enter_context(tc.tile_pool(name="const", bufs=1))
        main = ctx.enter_context(tc.tile_pool(name="main", bufs=1))
        scr = ctx.enter_context(tc.tile_pool(name="scr", bufs=1))
        pp = ctx.enter_context(tc.tile_pool(name="pp", bufs=7, space="PSUM"))
        pt = ctx.enter_context(tc.tile_pool(name="pt", bufs=1, space="PSUM"))
        dram = ctx.enter_context(tc.tile_pool(name="dram", bufs=1, space="DRAM"))

        # --- collective warm-up -------------------------------------------
        # The first collective pays the cross-core init barrier plus mesh
        # setup (~30-40us observed). Fire a tiny throwaway AllGather first
        # so it absorbs that cost while the input DMAs / conv1 run.
        warm_in = dram.tile([8, 32], F32, name="warm_in", tag="warm_in")
        warm_out = dram.tile(
            [N_CORES, 8, 32], F32, name="warm_out", tag="warm_out",
            addr_space="Shared",
        )
        nc.gpsimd.collective_compute(
            "AllGather",
            ALU.bypass,
            ins=[warm_in[:].opt()],
            outs=[warm_out[:].opt()],
            replica_groups=[list(range(N_CORES))],
        )

        # --- params + input, in critical-path order -----------------------
        # Chain the big DMAs so conv1's first group (w1 + x image 0 rows
        # 0..34) lands first instead of all transfers sharing bandwidth.
        from concourse.bass import _add_dep_helper

        w1_sb = const.tile([C, 9 * C], BF16, name="w1_sb", tag="w1_sb")
        prev = nc.scalar.dma_start(w1_sb[:], w1_d[:])

        xp_sb = []
        for b in range(B_SH):
            t = main.tile([C, HP, WP], BF16, name=f"xp{b}", tag=f"xp{b}")
            if b == 0:
                # split image 0 so conv1's first tap-major group (tiles 0-3,
                # padded rows 0..34) unblocks early
                d = nc.scalar.dma_start(t[:, 0:35, :], xp_d[b][:, 0:35, :])
                _add_dep_helper(d.ins, prev.ins, sync=True, reason="dma priority chain")
                d2 = nc.scalar.dma_start(t[:, 35:, :], xp_d[b][:, 35:, :])
                _add_dep_helper(d2.ins, d.ins, sync=True, reason="dma priority chain")
                prev = d2
            else:
                d = nc.scalar.dma_start(t[:], xp_d[b])
                _add_dep_helper(d.ins, prev.ins, sync=True, reason="dma priority chain")
                prev = d
            xp_sb.append(t)

        w2_sb = const.tile([C, 9 * C], BF16, name="w2_sb", tag="w2_sb")
        bn_par = {}
        for nm in ("g1", "b1", "g2", "b2"):
            bn_par[nm] = const.tile([C, 1], F32, name=f"{nm}_sb", tag=f"{nm}_sb")

        y1p = []  # conv1 raw output, padded buffer (later normalized in place)
        for b in range(B_SH):
            t = main.tile([C, HP, WP], BF16, name=f"y1p{b}", tag=f"y1p{b}")
            # zero the 1-px frame (interior is fully overwritten by conv1)
            nc.gpsimd.memset(t[:, 0, :], 0.0)
            nc.gpsimd.memset(t[:, HP - 1, :], 0.0)
            nc.gpsimd.memset(t[:, :, 0], 0.0)
            nc.gpsimd.memset(t[:, :, WP - 1], 0.0)
            y1p.append(t)

        y2 = []
        for b in range(B_SH):
            t = main.tile([C, H, W], BF16, name=f"y2_{b}", tag=f"y2_{b}")
            y2.append(t)

        # prewarm the ACT sqrt table set (Copy/Relu ride along in every set)
        warm_act = scr.tile([C, 1], F32, name="warm_act", tag="warm_act")
        nc.vector.memset(warm_act[:], 1.0)
        nc.scalar.activation(warm_act[:], warm_act[:], AF.Sqrt)

        # identity for PE-side transposes of the BN stat vectors
        id128 = const.tile([C, C], F32, name="id128", tag="id128")
        masks.make_identity(nc, id128[:])

        # [0, EPS] per-channel column pair folded into the mean/ex2 scaling
        epscol = const.tile([C, 2], F32, name="epscol", tag="epscol")
        nc.vector.memset(epscol[:, 0:1], 0.0)
        nc.vector.memset(epscol[:, 1:2], EPS)

        # per-tile BN partials: [:, 0, t] = sum, [:, 1, t] = sumsq
        st1 = scr.tile([C, 2, NT], F32, name="st1", tag="st1")
        st2 = scr.tile([C, 2, NT], F32, name="st2", tag="st2")

        sq_scr = scr.tile([C, ROWS, W], F32, name="sq_scr", tag="sq_scr")

        def conv(x_tiles, w_sb, writer):
            # tap-major within groups of GRP tiles: one weight load per tap
            # per group (the dedupe pass removes the repeats), PSUM pool
            # (bufs=7) keeps the next group's matmuls going while this
            # group drains.
            for g0 in range(0, NT, GRP):
                idxs = list(range(g0, min(g0 + GRP, NT)))
                pss = {i: pp.tile([C, ROWS, W], F32, name="ps", tag="ps") for i in idxs}
                for tap in range(9):
                    ky, kx = divmod(tap, 3)
                    for idx in idxs:
                        b, t = divmod(idx, TPB)
                        h0 = t * ROWS
                        rhs = x_tiles[b][:, h0 + ky : h0 + ky + ROWS, kx : kx + W]
                        nc.tensor.matmul(
                            pss[idx][:],
                            w_sb[:, tap * C : (tap + 1) * C],
                            rhs,
                            start=(tap == 0),
                            stop=(tap == 8),
                        )
                for idx in idxs:
                    b, t = divmod(idx, TPB)
                    writer(b, t, idx, pss[idx])

        def stat_writer(dst_of, st_tile):
            def w(b, t, idx, ps):
                # PSUM -> SBUF drain + per-channel sum on ScalarE
                dst = dst_of(b, t)
                nc.scalar.activation(
                    dst, ps[:], AF.Copy, accum_out=st_tile[:, 0, idx : idx + 1]
                )
                # sum of squares on VectorE, from the SBUF copy (PSUM has
                # only one DVE read port; tensor_tensor_reduce faults on hw)
                nc.vector.scalar_tensor_tensor(
                    sq_scr[:],
                    dst,
                    1.0,
                    dst,
                    ALU.mult,
                    ALU.mult,
                    accum_out=st_tile[:, 1, idx : idx + 1],
                )

            return w

        def sync_stats(st_tile, tag):
            # reduce the 28 per-tile partials, transpose [C,2]->[2,C] on the
            # (idle) PE so the collective-input DMA is 2 contiguous lines,
            # AllGather, read back as [16,C], transpose back to [C,(8,2)]
            # and reduce over cores.
            loc = scr.tile([C, 2], F32, name=f"loc{tag}", tag=f"loc{tag}")
            nc.vector.tensor_reduce(loc[:], st_tile[:], mybir.AxisListType.X, ALU.add)
            ps_send = pt.tile([2, C], F32, name=f"tps{tag}", tag="tp")
            nc.tensor.transpose(ps_send[:], loc[:], id128[:])
            loc_t = scr.tile([2, C], F32, name=f"loct{tag}", tag=f"loct{tag}")
            nc.scalar.activation(loc_t[:], ps_send[:], AF.Copy)
            cc_in = dram.tile([2, C], F32, name=f"ccin{tag}", tag=f"ccin{tag}")
            cc_out = dram.tile(
                [2 * N_CORES, C], F32, name=f"ccout{tag}", tag=f"ccout{tag}",
                addr_space="Shared",
            )
            nc.sync.dma_start(cc_in[:], loc_t[:])
            nc.gpsimd.collective_compute(
                "AllGather",
                ALU.bypass,
                ins=[cc_in[:].opt()],
                outs=[cc_out[:].opt()],
                replica_groups=[list(range(N_CORES))],
            )
            graw = scr.tile([2 * N_CORES, C], F32, name=f"graw{tag}", tag=f"graw{tag}")
            nc.sync.dma_start(graw[:], cc_out[:])
            ps_recv = pt.tile([C, N_CORES, 2], F32, name=f"tpr{tag}", tag="tp")
            nc.tensor.transpose(
                ps_recv[:], graw[:], id128[0 : 2 * N_CORES, 0 : 2 * N_CORES]
            )
            glob = scr.tile([C, 2], F32, name=f"glob{tag}", tag=f"glob{tag}")
            nc.vector.tensor_reduce(
                glob[:], ps_recv[:].transpose([0, 2, 1]), mybir.AxisListType.X, ALU.add
            )
            return glob

        def bn_coef(glob, g_sb, b_sb, tag):
            cf = scr.tile([C, 8], F32, name=f"cf{tag}", tag=f"cf{tag}")
            col = lambda i: cf[:, i : i + 1]
            msq, veps, s0, r0, tnw, inv, scl, bia = (col(i) for i in range(8))
            me = scr.tile([C, 2], F32, name=f"me{tag}", tag=f"me{tag}")
            mean, e2e = me[:, 0:1], me[:, 1:2]
            # me = glob/N + [0, EPS]  ->  [mean, ex2+EPS]
            nc.vector.scalar_tensor_tensor(
                me[:], glob[:], 1.0 / N_GLOB, epscol[:], ALU.mult, ALU.add
            )
            nc.vector.tensor_tensor(msq, mean, mean, ALU.mult)
            nc.vector.tensor_tensor(veps, e2e, msq, ALU.subtract)
            # rsqrt(veps): ACT sqrt (low precision) + one Newton step, then
            # exact-ish DVE reciprocal. scl folds in the Newton 1/2.
            nc.scalar.activation(s0, veps, AF.Sqrt)
            nc.vector.reciprocal(r0, s0)
            nc.vector.scalar_tensor_tensor(tnw, veps, r0, s0, ALU.mult, ALU.add)
            nc.vector.reciprocal(inv, tnw)
            nc.vector.tensor_scalar(scl, inv, g_sb[:], 2.0, ALU.mult, ALU.mult)
            # bia = beta - mean*scl
            nc.vector.tensor_scalar(bia, mean, scl, -1.0, ALU.mult, ALU.mult)
            nc.vector.tensor_tensor(bia, bia, b_sb[:], ALU.add)
            return scl, bia

        # ============ conv1 + BN1 stats ============
        conv(
            xp_sb,
            w1_sb,
            stat_writer(
                lambda b, t: y1p[b][:, 1 + t * ROWS : 1 + (t + 1) * ROWS, 1 : 1 + W],
                st1,
            ),
        )
        # deferred: conv2 weights + BN params (not needed until after conv1)
        nc.sync.dma_start(w2_sb[:], w2_d[:])
        for nm, dram_t in (("g1", g1_d), ("b1", b1_d), ("g2", g2_d), ("b2", b2_d)):
            nc.sync.dma_start(bn_par[nm][:], dram_t[:])

        glob1 = sync_stats(st1, "1")
        scl1, bia1 = bn_coef(glob1, bn_par["g1"], bn_par["b1"], "1")

        # normalize + relu, in place (interior only; border stays zero).
        # image 0 is split so conv2's first tap-major group (rows 0..33)
        # unblocks asap.
        norm_chunks = [(0, 0, 34), (0, 34, 22), (1, 0, 56), (2, 0, 56), (3, 0, 56)]
        for b, r0, nr in norm_chunks:
            itr = y1p[b][:, 1 + r0 : 1 + r0 + nr, 1 : 1 + W]
            nc.scalar.activation(itr, itr, AF.Relu, bias=bia1, scale=scl1)

        # ============ conv2 + BN2 stats ============
        conv(
            y1p,
            w2_sb,
            stat_writer(
                lambda b, t: y2[b][:, t * ROWS : (t + 1) * ROWS, :],
                st2,
            ),
        )
        glob2 = sync_stats(st2, "2")
        scl2, bia2 = bn_coef(glob2, bn_par["g2"], bn_par["b2"], "2")

        # ============ final: relu(y2*scl2 + bia2 + x) ============
        # bf16 end to end; chunked so DVE / ACT / DMA-out pipeline, output
        # DMAs alternate between two queues.
        FH = H // 4
        for b in range(B_SH):
            for quarter in range(4):
                r0 = quarter * FH
                ys = y2[b][:, r0 : r0 + FH, :]
                xs = xp_sb[b][:, 1 + r0 : 1 + r0 + FH, 1 : 1 + W]
                nc.vector.scalar_tensor_tensor(ys, ys, scl2, xs, ALU.mult, ALU.add)
                nc.scalar.activation(ys, ys, AF.Relu, bias=bia2, scale=1.0)
                eng = nc.sync if (b * 4 + quarter) % 2 == 0 else nc.gpsimd
                eng.dma_start(out_d[b][:, r0 : r0 + FH, :], ys)

    _dedupe_ldweights(nc)
    return nc


@functools.lru_cache(maxsize=1)
def get_nc():
    nc = _build()
    nc.compile()
    return nc


def make_in_maps(x, w1, gamma1, beta1, w2, gamma2, beta2):
    x = np.ascontiguousarray(np.asarray(x, dtype=np.float32))
    xp = np.zeros((B, C, HP, WP), ml_dtypes.bfloat16)
    xp[:, :, 1 : 1 + H, 1 : 1 + W] = x.astype(ml_dtypes.bfloat16)
    # w[o,i,ky,kx] -> [i, (ky,kx,o)] so tap t's lhsT slice is [C_in, C_out]
    w1t = np.ascontiguousarray(
        np.asarray(w1, np.float32).transpose(1, 2, 3, 0)
    ).reshape(C, 9 * C).astype(ml_dtypes.bfloat16)
    w2t = np.ascontiguousarray(
        np.asarray(w2, np.float32).transpose(1, 2, 3, 0)
    ).reshape(C, 9 * C).astype(ml_dtypes.bfloat16)
    g1 = np.ascontiguousarray(np.asarray(gamma1, np.float32).reshape(C, 1))
    b1 = np.ascontiguousarray(np.asarray(beta1, np.float32).reshape(C, 1))
    g2 = np.ascontiguousarray(np.asarray(gamma2, np.float32).reshape(C, 1))
    b2 = np.ascontiguousarray(np.asarray(beta2, np.float32).reshape(C, 1))
    maps = []
    for i in range(N_CORES):
        maps.append(
            {
                "xp": np.ascontiguousarray(xp[i * B_SH : (i + 1) * B_SH]),
                "w1t": w1t,
                "w2t": w2t,
                "g1": g1,
                "b1": b1,
                "g2": g2,
                "b2": b2,
            }
        )
    return maps


def run(in_maps, trace=False, **kwargs):
    nc = get_nc()
    return run_bass_kernel_spmd(
        nc, in_maps, core_ids=list(range(N_CORES)), trace=trace, **kwargs
    )


def kernel(x, w1, gamma1, beta1, w2, gamma2, beta2):
    maps = make_in_maps(x, w1, gamma1, beta1, w2, gamma2, beta2)
    res = run(maps)
    out = np.concatenate([res.results[i]["out"] for i in range(N_CORES)], axis=0)
    return np.ascontiguousarray(out.astype(np.float32))


# revision 10
# speedup vs baseline: 1.1310x; 1.1310x over previous
"""ResNet BasicBlock (conv3x3-BN-ReLU-conv3x3-BN-add-ReLU) on 8 TRN2 NeuronCores.

Data-parallel over batch (4 images per core). Convs are implicit GEMM on the
TensorEngine: 9 shifted-window bf16 matmuls accumulated per PSUM row-tile,
issued tap-major over groups of 4 tiles so one LDWEIGHTS serves 4 matmuls
(a post-legalize pass drops the redundant weight loads). Training-mode BN is
exact sync-BN: per-core (sum, sumsq) partials are transposed on the PE into a
[2,C] tile so the collective-input DMA is 2 contiguous lines instead of a
128-partition scatter (which cost ~8us of doorbell latency), AllGathered, and
transposed back for the per-channel coefficient math. A throwaway AllGather at
kernel start absorbs the cross-core collective-init barrier; a dummy Sqrt
preloads the ACT table set. The BN rsqrt runs as ACT Sqrt + one Newton step +
DVE reciprocal. conv2 output, the fused relu(scale*y + bias + x) epilogue and
the output DMA all run in bf16 (host upcasts), halving DVE/ACT/HBM cost of the
tail; output DMAs alternate between two queues.
"""

import functools
from contextlib import ExitStack

import ml_dtypes
import numpy as np

from concourse import bacc, bass, masks, mybir, tile
from concourse.bass_utils import run_bass_kernel_spmd

F32 = mybir.dt.float32
BF16 = mybir.dt.bfloat16
AF = mybir.ActivationFunctionType
ALU = mybir.AluOpType

N_CORES = 8
B, C, H, W = 32, 128, 56, 56
B_SH = B // N_CORES           # 4 images per core
HP, WP = H + 2, W + 2         # 58 (zero-padded)
ROWS = 8                      # output rows per conv tile
TPB = H // ROWS               # 7 tiles per image
NT = B_SH * TPB               # 28 tiles per conv per core
# tap-major group sizes: small at the start (conv can begin on a sliver of
# input / normalized rows) and at the end (the last tile's BN-stat drain is
# the sync-BN critical path)
GROUPS = [1, 1, 2, 4, 4, 4, 4, 4, 2, 1, 1]
assert sum(GROUPS) == NT
N_GLOB = B * H * W            # BN sample count
EPS = 1e-5


def _dedupe_ldweights(nc):
    """Drop InstLdweights that reload the exact weights already resident in
    the PE array (tap-major matmul groups make runs of 4 identical loads).
    Only loads with no sync_info are dropped; any other PE instruction
    conservatively invalidates the tracked state."""
    PE = mybir.EngineType.PE
    for f in nc.m.functions:
        for bb in f.blocks:
            il = bb.instructions
            keep = []
            last_sig = None
            for inst in il:
                tn = type(inst).__name__
                if tn == "InstLdweights":
                    ap = inst.ins[0]
                    sig = (
                        str(ap),
                        str(inst.perf_mode),
                        str(inst.is_transpose),
                        str(inst.tile_position),
                    )
                    si = inst.sync_info
                    clean = si is None or (
                        len(si.on_wait) == 0 and len(si.on_update) == 0
                    )
                    if clean and sig == last_sig:
                        continue  # redundant reload of resident weights
                    last_sig = sig
                    keep.append(inst)
                    continue
                keep.append(inst)
                if getattr(inst, "engine", None) == PE:
                    if tn == "InstMatmult" and inst.ldweights is False and not inst.is_transpose:
                        pass  # consumes resident weights, does not change them
                    else:
                        last_sig = None
            if len(keep) != len(il):
                il.clear()
                il.extend(keep)


def _build():
    nc = bacc.Bacc(
        "TRN2",
        target_bir_lowering=False,
        debug=False,
        enable_asserts=False,
        num_devices=N_CORES,
    )

    xp_d = nc.dram_tensor("xp", [B_SH, C, HP, WP], BF16, kind="ExternalInput")
    w1_d = nc.dram_tensor("w1t", [C, 9 * C], BF16, kind="ExternalInput")
    w2_d = nc.dram_tensor("w2t", [C, 9 * C], BF16, kind="ExternalInput")
    g1_d = nc.dram_tensor("g1", [C, 1], F32, kind="ExternalInput")
    b1_d = nc.dram_tensor("b1", [C, 1], F32, kind="ExternalInput")
    g2_d = nc.dram_tensor("g2", [C, 1], F32, kind="ExternalInput")
    b2_d = nc.dram_tensor("b2", [C, 1], F32, kind="ExternalInput")
    out_d = nc.dram_tensor("out", [B_SH, C, H, W], BF16, kind="ExternalOutput")

    with tile.TileContext(nc) as tc, ExitStack() as ctx:
        const = ctx.enter_context(tc.tile_pool(name="const", bufs=1))
        main = ctx.enter_context(tc.tile_pool(name="main", bufs=1))
        scr = ctx.enter_context(tc.tile_pool(name="scr", bufs=1))
        pp = ctx.enter_context(tc.tile_pool(name="pp", bufs=7, space="PSUM"))
        pt = ctx.enter_context(tc.tile_pool(name="pt", bufs=1, space="PSUM"))
        dram = ctx.enter_context(tc.tile_pool(name="dram", bufs=1, space="DRAM"))

        # --- collective warm-up -------------------------------------------
        # The first collective pays the cross-core init barrier plus mesh
        # setup (~30-40us observed). Fire a tiny throwaway AllGather first
        # so it absorbs that cost while the input DMAs / conv1 run.
        warm_in = dram.tile([8, 32], F32, name="warm_in", tag="warm_in")
        warm_out = dram.tile(
            [N_CORES, 8, 32], F32, name="warm_out", tag="warm_out",
            addr_space="Shared",
        )
        nc.gpsimd.collective_compute(
            "AllGather",
            ALU.bypass,
            ins=[warm_in[:].opt()],
            outs=[warm_out[:].opt()],
            replica_groups=[list(range(N_CORES))],
        )

        # --- params + input, in critical-path order -----------------------
        # w1 rides the sync queue in parallel with the x chain on the scalar
        # queue; image 0 is split so conv1's first (1-tile) groups unblock on
        # a sliver of input. w2 + BN params are chained after the last image
        # so they don't steal bandwidth from conv1's critical path.
        from concourse.bass import _add_dep_helper

        w1_sb = const.tile([C, 9 * C], BF16, name="w1_sb", tag="w1_sb")
        nc.sync.dma_start(w1_sb[:], w1_d[:])

        xp_sb = []
        prev = None
        for b in range(B_SH):
            t = main.tile([C, HP, WP], BF16, name=f"xp{b}", tag=f"xp{b}")
            if b == 0:
                for r0, r1 in ((0, 11), (11, 19), (19, 35), (35, HP)):
                    d = nc.scalar.dma_start(t[:, r0:r1, :], xp_d[b][:, r0:r1, :])
                    if prev is not None:
                        _add_dep_helper(
                            d.ins, prev.ins, sync=True, reason="dma priority chain"
                        )
                    prev = d
            else:
                d = nc.scalar.dma_start(t[:], xp_d[b])
                _add_dep_helper(d.ins, prev.ins, sync=True, reason="dma priority chain")
                prev = d
            xp_sb.append(t)

        w2_sb = const.tile([C, 9 * C], BF16, name="w2_sb", tag="w2_sb")
        d = nc.scalar.dma_start(w2_sb[:], w2_d[:])
        _add_dep_helper(d.ins, prev.ins, sync=True, reason="dma priority chain")
        prev = d
        bn_par = {}
        for nm, dram_t in (("g1", g1_d), ("b1", b1_d), ("g2", g2_d), ("b2", b2_d)):
            bn_par[nm] = const.tile([C, 1], F32, name=f"{nm}_sb", tag=f"{nm}_sb")
            d = nc.scalar.dma_start(bn_par[nm][:], dram_t[:])
            _add_dep_helper(d.ins, prev.ins, sync=True, reason="dma priority chain")
            prev = d

        y1p = []  # conv1 raw output, padded buffer (later normalized in place)
        for b in range(B_SH):
            t = main.tile([C, HP, WP], BF16, name=f"y1p{b}", tag=f"y1p{b}")
            # zero the 1-px frame (interior is fully overwritten by conv1)
            nc.gpsimd.memset(t[:, 0, :], 0.0)
            nc.gpsimd.memset(t[:, HP - 1, :], 0.0)
            nc.gpsimd.memset(t[:, :, 0], 0.0)
            nc.gpsimd.memset(t[:, :, WP - 1], 0.0)
            y1p.append(t)

        y2 = []
        for b in range(B_SH):
            t = main.tile([C, H, W], BF16, name=f"y2_{b}", tag=f"y2_{b}")
            y2.append(t)

        # prewarm the ACT sqrt table set (Copy/Relu ride along in every set)
        warm_act = scr.tile([C, 1], F32, name="warm_act", tag="warm_act")
        nc.vector.memset(warm_act[:], 1.0)
        nc.scalar.activation(warm_act[:], warm_act[:], AF.Sqrt)

        # identity for PE-side transposes of the BN stat vectors, plus a bf16
        # copy (and a diag(scl2) tile) for the PE-side final epilogue
        id128 = const.tile([C, C], F32, name="id128", tag="id128")
        masks.make_identity(nc, id128[:])
        id_bf = const.tile([C, C], BF16, name="id_bf", tag="id_bf")
        nc.scalar.activation(id_bf[:], id128[:], AF.Copy)
        diag_bf = const.tile([C, C], BF16, name="diag_bf", tag="diag_bf")

        # [0, EPS] per-channel column pair folded into the mean/ex2 scaling
        epscol = const.tile([C, 2], F32, name="epscol", tag="epscol")
        nc.vector.memset(epscol[:, 0:1], 0.0)
        nc.vector.memset(epscol[:, 1:2], EPS)

        # per-tile BN partials: [:, 0, t] = sum, [:, 1, t] = sumsq
        st1 = scr.tile([C, 2, NT], F32, name="st1", tag="st1")
        st2 = scr.tile([C, 2, NT], F32, name="st2", tag="st2")

        sq_scr = scr.tile([C, ROWS, W], F32, name="sq_scr", tag="sq_scr")

        def conv(x_tiles, w_sb, writer):
            # tap-major within groups: one weight load per tap per group
            # (the dedupe pass removes the repeats), PSUM pool (bufs=7)
            # keeps the next group's matmuls going while this group drains.
            g0 = 0
            for gsz in GROUPS:
                idxs = list(range(g0, g0 + gsz))
                g0 += gsz
                pss = {i: pp.tile([C, ROWS, W], F32, name="ps", tag="ps") for i in idxs}
                for tap in range(9):
                    ky, kx = divmod(tap, 3)
                    for idx in idxs:
                        b, t = divmod(idx, TPB)
                        h0 = t * ROWS
                        rhs = x_tiles[b][:, h0 + ky : h0 + ky + ROWS, kx : kx + W]
                        nc.tensor.matmul(
                            pss[idx][:],
                            w_sb[:, tap * C : (tap + 1) * C],
                            rhs,
                            start=(tap == 0),
                            stop=(tap == 8),
                        )
                for idx in idxs:
                    b, t = divmod(idx, TPB)
                    writer(b, t, idx, pss[idx])

        def stat_writer(dst_of, st_tile):
            def w(b, t, idx, ps):
                # PSUM -> SBUF drain + per-channel sum on ScalarE
                dst = dst_of(b, t)
                nc.scalar.activation(
                    dst, ps[:], AF.Copy, accum_out=st_tile[:, 0, idx : idx + 1]
                )
                # sum of squares on VectorE, from the SBUF copy (PSUM has
                # only one DVE read port; tensor_tensor_reduce faults on hw)
                nc.vector.scalar_tensor_tensor(
                    sq_scr[:],
                    dst,
                    1.0,
                    dst,
                    ALU.mult,
                    ALU.mult,
                    accum_out=st_tile[:, 1, idx : idx + 1],
                )

            return w

        def sync_stats(st_tile, tag):
            # reduce the 28 per-tile partials, transpose [C,2]->[2,C] on the
            # (idle) PE so the collective-input DMA is 2 contiguous lines,
            # AllGather, read back as [16,C], transpose back to [C,(8,2)]
            # and reduce over cores.
            loc = scr.tile([C, 2], F32, name=f"loc{tag}", tag=f"loc{tag}")
            nc.vector.tensor_reduce(loc[:], st_tile[:], mybir.AxisListType.X, ALU.add)
            ps_send = pt.tile([2, C], F32, name=f"tps{tag}", tag="tp")
            nc.tensor.transpose(ps_send[:], loc[:], id128[:])
            loc_t = scr.tile([2, C], F32, name=f"loct{tag}", tag=f"loct{tag}")
            nc.scalar.activation(loc_t[:], ps_send[:], AF.Copy)
            cc_in = dram.tile([2, C], F32, name=f"ccin{tag}", tag=f"ccin{tag}")
            cc_out = dram.tile(
                [2 * N_CORES, C], F32, name=f"ccout{tag}", tag=f"ccout{tag}",
                addr_space="Shared",
            )
            nc.sync.dma_start(cc_in[:], loc_t[:])
            nc.gpsimd.collective_compute(
                "AllGather",
                ALU.bypass,
                ins=[cc_in[:].opt()],
                outs=[cc_out[:].opt()],
                replica_groups=[list(range(N_CORES))],
            )
            graw = scr.tile([2 * N_CORES, C], F32, name=f"graw{tag}", tag=f"graw{tag}")
            nc.sync.dma_start(graw[:], cc_out[:])
            ps_recv = pt.tile([C, N_CORES, 2], F32, name=f"tpr{tag}", tag="tp")
            nc.tensor.transpose(
                ps_recv[:], graw[:], id128[0 : 2 * N_CORES, 0 : 2 * N_CORES]
            )
            glob = scr.tile([C, 2], F32, name=f"glob{tag}", tag=f"glob{tag}")
            nc.vector.tensor_reduce(
                glob[:], ps_recv[:].transpose([0, 2, 1]), mybir.AxisListType.X, ALU.add
            )
            return glob

        def bn_coef(glob, g_sb, b_sb, tag):
            cf = scr.tile([C, 8], F32, name=f"cf{tag}", tag=f"cf{tag}")
            col = lambda i: cf[:, i : i + 1]
            msq, veps, s0, r0, tnw, inv, scl, bia = (col(i) for i in range(8))
            me = scr.tile([C, 2], F32, name=f"me{tag}", tag=f"me{tag}")
            mean, e2e = me[:, 0:1], me[:, 1:2]
            # me = glob/N + [0, EPS]  ->  [mean, ex2+EPS]
            nc.vector.scalar_tensor_tensor(
                me[:], glob[:], 1.0 / N_GLOB, epscol[:], ALU.mult, ALU.add
            )
            nc.vector.tensor_tensor(msq, mean, mean, ALU.mult)
            nc.vector.tensor_tensor(veps, e2e, msq, ALU.subtract)
            # rsqrt(veps): ACT sqrt (low precision) + one Newton step, then
            # exact-ish DVE reciprocal. scl folds in the Newton 1/2.
            nc.scalar.activation(s0, veps, AF.Sqrt)
            nc.vector.reciprocal(r0, s0)
            nc.vector.scalar_tensor_tensor(tnw, veps, r0, s0, ALU.mult, ALU.add)
            nc.vector.reciprocal(inv, tnw)
            nc.vector.tensor_scalar(scl, inv, g_sb[:], 2.0, ALU.mult, ALU.mult)
            # bia = beta - mean*scl
            nc.vector.tensor_scalar(bia, mean, scl, -1.0, ALU.mult, ALU.mult)
            nc.vector.tensor_tensor(bia, bia, b_sb[:], ALU.add)
            return scl, bia

        # ============ conv1 + BN1 stats ============
        conv(
            xp_sb,
            w1_sb,
            stat_writer(
                lambda b, t: y1p[b][:, 1 + t * ROWS : 1 + (t + 1) * ROWS, 1 : 1 + W],
                st1,
            ),
        )
        glob1 = sync_stats(st1, "1")
        scl1, bia1 = bn_coef(glob1, bn_par["g1"], bn_par["b1"], "1")

        # normalize + relu, in place (interior only; border stays zero).
        # image 0 is split to track conv2's small leading groups.
        norm_chunks = [
            (0, 0, 10), (0, 10, 8), (0, 18, 16), (0, 34, 22),
            (1, 0, 56), (2, 0, 56), (3, 0, 56),
        ]
        for b, r0, nr in norm_chunks:
            itr = y1p[b][:, 1 + r0 : 1 + r0 + nr, 1 : 1 + W]
            nc.scalar.activation(itr, itr, AF.Relu, bias=bia1, scale=scl1)

        # ============ conv2 + BN2 stats ============
        conv(
            y1p,
            w2_sb,
            stat_writer(
                lambda b, t: y2[b][:, t * ROWS : (t + 1) * ROWS, :],
                st2,
            ),
        )
        glob2 = sync_stats(st2, "2")
        scl2, bia2 = bn_coef(glob2, bn_par["g2"], bn_par["b2"], "2")

        # ============ final: relu(y2*scl2 + bia2 + x) ============
        # scl2*y2 + x is accumulated on the (idle) TensorEngine: a diag(scl2)
        # matmul plus an identity matmul per 8-row tile. The relu(+bia2)
        # drains split between ACT and DVE (alternating tiles) writing bf16
        # in place over y2; output DMAs go in 28-row chunks on two queues.
        nc.vector.tensor_scalar(diag_bf[:], id128[:], scl2, None, ALU.mult)
        for b in range(B_SH):
            for t in range(TPB):
                r0 = t * ROWS
                ys = y2[b][:, r0 : r0 + ROWS, :]
                xs = xp_sb[b][:, 1 + r0 : 1 + r0 + ROWS, 1 : 1 + W]
                ps = pp.tile([C, ROWS, W], F32, name="ps", tag="ps")
                nc.tensor.matmul(ps[:], diag_bf[:], ys, start=True, stop=False)
                nc.tensor.matmul(ps[:], id_bf[:], xs, start=False, stop=True)
                if t % 2 == 0:
                    nc.scalar.activation(ys, ps[:], AF.Relu, bias=bia2, scale=1.0)
                else:
                    nc.vector.tensor_scalar(
                        ys, ps[:], bia2, 0.0, ALU.add, ALU.max
                    )
                if t % 4 == 3 or t == TPB - 1:
                    # rows [lo, hi) of this image are final -> ship them
                    lo = (t // 4) * 4 * ROWS
                    hi = (t + 1) * ROWS
                    eng = nc.sync if (t // 4) % 2 == 0 else nc.gpsimd
                    eng.dma_start(
                        out_d[b][:, lo:hi, :], y2[b][:, lo:hi, :]
                    )

    _dedupe_ldweights(nc)
    return nc


@functools.lru_cache(maxsize=1)
def get_nc():
    nc = _build()
    nc.compile()
    return nc


def make_in_maps(x, w1, gamma1, beta1, w2, gamma2, beta2):
    x = np.ascontiguousarray(np.asarray(x, dtype=np.float32))
    xp = np.zeros((B, C, HP, WP), ml_dtypes.bfloat16)
    xp[:, :, 1 : 1 + H, 1 : 1 + W] = x.astype(ml_dtypes.bfloat16)
    # w[o,i,ky,kx] -> [i, (ky,kx,o)] so tap t's lhsT slice is [C_in, C_out]
    w1t = np.ascontiguousarray(
        np.asarray(w1, np.float32).transpose(1, 2, 3, 0)
    ).reshape(C, 9 * C).astype(ml_dtypes.bfloat16)
    w2t = np.ascontiguousarray(
        np.asarray(w2, np.float32).transpose(1, 2, 3, 0)
    ).reshape(C, 9 * C).astype(ml_dtypes.bfloat16)
    g1 = np.ascontiguousarray(np.asarray(gamma1, np.float32).reshape(C, 1))
    b1 = np.ascontiguousarray(np.asarray(beta1, np.float32).reshape(C, 1))
    g2 = np.ascontiguousarray(np.asarray(gamma2, np.float32).reshape(C, 1))
    b2 = np.ascontiguousarray(np.asarray(beta2, np.float32).reshape(C, 1))
    maps = []
    for i in range(N_CORES):
        maps.append(
            {
                "xp": np.ascontiguousarray(xp[i * B_SH : (i + 1) * B_SH]),
                "w1t": w1t,
                "w2t": w2t,
                "g1": g1,
                "b1": b1,
                "g2": g2,
                "b2": b2,
            }
        )
    return maps


def run(in_maps, trace=False, **kwargs):
    nc = get_nc()
    return run_bass_kernel_spmd(
        nc, in_maps, core_ids=list(range(N_CORES)), trace=trace, **kwargs
    )


def kernel(x, w1, gamma1, beta1, w2, gamma2, beta2):
    maps = make_in_maps(x, w1, gamma1, beta1, w2, gamma2, beta2)
    res = run(maps)
    out = np.concatenate([res.results[i]["out"] for i in range(N_CORES)], axis=0)
    return np.ascontiguousarray(out.astype(np.float32))


# revision 12
# speedup vs baseline: 1.1641x; 1.0293x over previous
"""ResNet BasicBlock (conv3x3-BN-ReLU-conv3x3-BN-add-ReLU) on 8 TRN2 NeuronCores.

Data-parallel over batch (4 images per core). Convs are implicit GEMM on the
TensorEngine: 9 shifted-window bf16 matmuls accumulated per PSUM row-tile,
issued tap-major over groups of 4 tiles so one LDWEIGHTS serves 4 matmuls
(a post-legalize pass drops the redundant weight loads). Training-mode BN is
exact sync-BN: per-core (sum, sumsq) partials are transposed on the PE into a
[2,C] tile so the collective-input DMA is 2 contiguous lines instead of a
128-partition scatter (which cost ~8us of doorbell latency), AllGathered, and
transposed back for the per-channel coefficient math. A throwaway AllGather at
kernel start absorbs the cross-core collective-init barrier; a dummy Sqrt
preloads the ACT table set. The BN rsqrt runs as ACT Sqrt + one Newton step +
DVE reciprocal. conv2 output, the fused relu(scale*y + bias + x) epilogue and
the output DMA all run in bf16 (host upcasts), halving DVE/ACT/HBM cost of the
tail; output DMAs alternate between two queues.
"""

import functools
from contextlib import ExitStack

import ml_dtypes
import numpy as np

from concourse import bacc, bass, masks, mybir, tile
from concourse.bass_utils import run_bass_kernel_spmd

F32 = mybir.dt.float32
BF16 = mybir.dt.bfloat16
AF = mybir.ActivationFunctionType
ALU = mybir.AluOpType

N_CORES = 8
B, C, H, W = 32, 128, 56, 56
B_SH = B // N_CORES           # 4 images per core
HP, WP = H + 2, W + 2         # 58 (zero-padded)
ROWS = 8                      # output rows per conv tile
TPB = H // ROWS               # 7 tiles per image
NT = B_SH * TPB               # 28 tiles per conv per core
# tap-major group sizes: small at the start (conv can begin on a sliver of
# input / normalized rows) and at the end (the last tile's BN-stat drain is
# the sync-BN critical path)
GROUPS = [1, 1, 2, 4, 4, 4, 4, 4, 2, 1, 1]
assert sum(GROUPS) == NT
N_GLOB = B * H * W            # BN sample count
EPS = 1e-5


def _dedupe_ldweights(nc):
    """Drop InstLdweights that reload the exact weights already resident in
    the PE array (tap-major matmul groups make runs of 4 identical loads).
    Only loads with no sync_info are dropped; any other PE instruction
    conservatively invalidates the tracked state."""
    PE = mybir.EngineType.PE
    for f in nc.m.functions:
        for bb in f.blocks:
            il = bb.instructions
            keep = []
            last_sig = None
            for inst in il:
                tn = type(inst).__name__
                if tn == "InstLdweights":
                    ap = inst.ins[0]
                    sig = (
                        str(ap),
                        str(inst.perf_mode),
                        str(inst.is_transpose),
                        str(inst.tile_position),
                    )
                    si = inst.sync_info
                    clean = si is None or (
                        len(si.on_wait) == 0 and len(si.on_update) == 0
                    )
                    if clean and sig == last_sig:
                        continue  # redundant reload of resident weights
                    last_sig = sig
                    keep.append(inst)
                    continue
                keep.append(inst)
                if getattr(inst, "engine", None) == PE:
                    if tn == "InstMatmult" and inst.ldweights is False and not inst.is_transpose:
                        pass  # consumes resident weights, does not change them
                    else:
                        last_sig = None
            if len(keep) != len(il):
                il.clear()
                il.extend(keep)


def _build():
    nc = bacc.Bacc(
        "TRN2",
        target_bir_lowering=False,
        debug=False,
        enable_asserts=False,
        num_devices=N_CORES,
    )

    xp_d = nc.dram_tensor("xp", [B_SH, C, HP, WP], BF16, kind="ExternalInput")
    w1_d = nc.dram_tensor("w1t", [C, 9 * C], BF16, kind="ExternalInput")
    w2_d = nc.dram_tensor("w2t", [C, 9 * C], BF16, kind="ExternalInput")
    g1_d = nc.dram_tensor("g1", [C, 1], F32, kind="ExternalInput")
    b1_d = nc.dram_tensor("b1", [C, 1], F32, kind="ExternalInput")
    g2_d = nc.dram_tensor("g2", [C, 1], F32, kind="ExternalInput")
    b2_d = nc.dram_tensor("b2", [C, 1], F32, kind="ExternalInput")
    out_d = nc.dram_tensor("out", [B_SH, C, H, W], BF16, kind="ExternalOutput")

    with tile.TileContext(nc) as tc, ExitStack() as ctx:
        const = ctx.enter_context(tc.tile_pool(name="const", bufs=1))
        main = ctx.enter_context(tc.tile_pool(name="main", bufs=1))
        scr = ctx.enter_context(tc.tile_pool(name="scr", bufs=1))
        pp = ctx.enter_context(tc.tile_pool(name="pp", bufs=7, space="PSUM"))
        pt = ctx.enter_context(tc.tile_pool(name="pt", bufs=1, space="PSUM"))
        dram = ctx.enter_context(tc.tile_pool(name="dram", bufs=1, space="DRAM"))

        # --- collective warm-up -------------------------------------------
        # The first collective pays the cross-core init barrier plus mesh
        # setup (~30-40us observed). Fire a tiny throwaway AllGather first
        # so it absorbs that cost while the input DMAs / conv1 run.
        warm_in = dram.tile([8, 32], F32, name="warm_in", tag="warm_in")
        warm_out = dram.tile(
            [N_CORES, 8, 32], F32, name="warm_out", tag="warm_out",
            addr_space="Shared",
        )
        nc.gpsimd.collective_compute(
            "AllGather",
            ALU.bypass,
            ins=[warm_in[:].opt()],
            outs=[warm_out[:].opt()],
            replica_groups=[list(range(N_CORES))],
        )

        # --- params + input, in critical-path order -----------------------
        # w1 rides the sync queue in parallel with the x chain on the scalar
        # queue; image 0 is split so conv1's first (1-tile) groups unblock on
        # a sliver of input. w2 + BN params are chained after the last image
        # so they don't steal bandwidth from conv1's critical path.
        from concourse.bass import _add_dep_helper

        # everything rides the sync queue (the scalar engine's queue is
        # blocked early by the auto-inserted ACT table loads), chained in
        # critical-path order: x0 sliver, w1, rest of x, then w2 + params.
        xp_sb = [
            main.tile([C, HP, WP], BF16, name=f"xp{b}", tag=f"xp{b}")
            for b in range(B_SH)
        ]
        w1_sb = const.tile([C, 9 * C], BF16, name="w1_sb", tag="w1_sb")
        w2_sb = const.tile([C, 9 * C], BF16, name="w2_sb", tag="w2_sb")
        bn_par = {}
        for nm in ("g1", "b1", "g2", "b2"):
            bn_par[nm] = const.tile([C, 1], F32, name=f"{nm}_sb", tag=f"{nm}_sb")

        prev = None

        def chain(d):
            nonlocal prev
            if prev is not None:
                _add_dep_helper(d.ins, prev.ins, sync=True, reason="dma priority chain")
            prev = d

        chain(nc.sync.dma_start(xp_sb[0][:, 0:10, :], xp_d[0][:, 0:10, :]))
        chain(nc.sync.dma_start(w1_sb[:], w1_d[:]))
        for r0, r1 in ((10, 19), (19, 35), (35, HP)):
            chain(nc.sync.dma_start(xp_sb[0][:, r0:r1, :], xp_d[0][:, r0:r1, :]))
        for b in range(1, B_SH):
            chain(nc.sync.dma_start(xp_sb[b][:], xp_d[b]))
        chain(nc.sync.dma_start(w2_sb[:], w2_d[:]))
        for nm, dram_t in (("g1", g1_d), ("b1", b1_d), ("g2", g2_d), ("b2", b2_d)):
            chain(nc.sync.dma_start(bn_par[nm][:], dram_t[:]))

        y1p = []  # conv1 raw output, padded buffer (later normalized in place)
        for b in range(B_SH):
            t = main.tile([C, HP, WP], BF16, name=f"y1p{b}", tag=f"y1p{b}")
            # zero the 1-px frame (interior is fully overwritten by conv1)
            nc.gpsimd.memset(t[:, 0, :], 0.0)
            nc.gpsimd.memset(t[:, HP - 1, :], 0.0)
            nc.gpsimd.memset(t[:, :, 0], 0.0)
            nc.gpsimd.memset(t[:, :, WP - 1], 0.0)
            y1p.append(t)

        y2 = []
        for b in range(B_SH):
            t = main.tile([C, H, W], BF16, name=f"y2_{b}", tag=f"y2_{b}")
            y2.append(t)

        # prewarm the ACT sqrt table set (Copy/Relu ride along in every set)
        warm_act = scr.tile([C, 1], F32, name="warm_act", tag="warm_act")
        nc.vector.memset(warm_act[:], 1.0)
        nc.scalar.activation(warm_act[:], warm_act[:], AF.Sqrt)

        # identity for PE-side transposes of the BN stat vectors, plus a bf16
        # copy (and a diag(scl2) tile) for the PE-side final epilogue
        id128 = const.tile([C, C], F32, name="id128", tag="id128")
        masks.make_identity(nc, id128[:])
        id_bf = const.tile([C, C], BF16, name="id_bf", tag="id_bf")
        nc.scalar.activation(id_bf[:], id128[:], AF.Copy)
        diag_bf = const.tile([C, C], BF16, name="diag_bf", tag="diag_bf")

        # [0, EPS] per-channel column pair folded into the mean/ex2 scaling
        epscol = const.tile([C, 2], F32, name="epscol", tag="epscol")
        nc.vector.memset(epscol[:, 0:1], 0.0)
        nc.vector.memset(epscol[:, 1:2], EPS)

        # per-tile BN partials: [:, 0, t] = sum, [:, 1, t] = sumsq
        st1 = scr.tile([C, 2, NT], F32, name="st1", tag="st1")
        st2 = scr.tile([C, 2, NT], F32, name="st2", tag="st2")

        sq_scr = scr.tile([C, ROWS, W], F32, name="sq_scr", tag="sq_scr")

        def conv(x_tiles, w_sb, writer):
            # tap-major within groups: one weight load per tap per group
            # (the dedupe pass removes the repeats), PSUM pool (bufs=7)
            # keeps the next group's matmuls going while this group drains.
            g0 = 0
            for gsz in GROUPS:
                idxs = list(range(g0, g0 + gsz))
                g0 += gsz
                pss = {i: pp.tile([C, ROWS, W], F32, name="ps", tag="ps") for i in idxs}
                for tap in range(9):
                    ky, kx = divmod(tap, 3)
                    for idx in idxs:
                        b, t = divmod(idx, TPB)
                        h0 = t * ROWS
                        rhs = x_tiles[b][:, h0 + ky : h0 + ky + ROWS, kx : kx + W]
                        nc.tensor.matmul(
                            pss[idx][:],
                            w_sb[:, tap * C : (tap + 1) * C],
                            rhs,
                            start=(tap == 0),
                            stop=(tap == 8),
                        )
                for idx in idxs:
                    b, t = divmod(idx, TPB)
                    writer(b, t, idx, pss[idx])

        def stat_writer(dst_of, st_tile):
            def w(b, t, idx, ps):
                # PSUM -> SBUF drain + per-channel sum on ScalarE
                dst = dst_of(b, t)
                nc.scalar.activation(
                    dst, ps[:], AF.Copy, accum_out=st_tile[:, 0, idx : idx + 1]
                )
                # sum of squares on VectorE, from the SBUF copy (PSUM has
                # only one DVE read port; tensor_tensor_reduce faults on hw)
                nc.vector.scalar_tensor_tensor(
                    sq_scr[:],
                    dst,
                    1.0,
                    dst,
                    ALU.mult,
                    ALU.mult,
                    accum_out=st_tile[:, 1, idx : idx + 1],
                )

            return w

        def sync_stats(st_tile, tag):
            # reduce the 28 per-tile partials, transpose [C,2]->[2,C] on the
            # (idle) PE so the collective-input DMA is 2 contiguous lines,
            # AllGather, read back as [16,C], transpose back to [C,(8,2)]
            # and reduce over cores.
            loc = scr.tile([C, 2], F32, name=f"loc{tag}", tag=f"loc{tag}")
            nc.vector.tensor_reduce(loc[:], st_tile[:], mybir.AxisListType.X, ALU.add)
            ps_send = pt.tile([2, C], F32, name=f"tps{tag}", tag="tp")
            nc.tensor.transpose(ps_send[:], loc[:], id128[:])
            loc_t = scr.tile([2, C], F32, name=f"loct{tag}", tag=f"loct{tag}")
            nc.scalar.activation(loc_t[:], ps_send[:], AF.Copy)
            cc_in = dram.tile([2, C], F32, name=f"ccin{tag}", tag=f"ccin{tag}")
            cc_out = dram.tile(
                [2 * N_CORES, C], F32, name=f"ccout{tag}", tag=f"ccout{tag}",
                addr_space="Shared",
            )
            nc.sync.dma_start(cc_in[:], loc_t[:])
            nc.gpsimd.collective_compute(
                "AllGather",
                ALU.bypass,
                ins=[cc_in[:].opt()],
                outs=[cc_out[:].opt()],
                replica_groups=[list(range(N_CORES))],
            )
            graw = scr.tile([2 * N_CORES, C], F32, name=f"graw{tag}", tag=f"graw{tag}")
            nc.sync.dma_start(graw[:], cc_out[:])
            ps_recv = pt.tile([C, N_CORES, 2], F32, name=f"tpr{tag}", tag="tp")
            nc.tensor.transpose(
                ps_recv[:], graw[:], id128[0 : 2 * N_CORES, 0 : 2 * N_CORES]
            )
            glob = scr.tile([C, 2], F32, name=f"glob{tag}", tag=f"glob{tag}")
            nc.vector.tensor_reduce(
                glob[:], ps_recv[:].transpose([0, 2, 1]), mybir.AxisListType.X, ALU.add
            )
            return glob

        NEWTON = False  # ACT Sqrt + DVE reciprocal lands well inside 2e-2

        def bn_coef(glob, g_sb, b_sb, tag):
            cf = scr.tile([C, 8], F32, name=f"cf{tag}", tag=f"cf{tag}")
            col = lambda i: cf[:, i : i + 1]
            msq, veps, s0, r0, tnw, inv, scl, bia = (col(i) for i in range(8))
            me = scr.tile([C, 2], F32, name=f"me{tag}", tag=f"me{tag}")
            mean, e2e = me[:, 0:1], me[:, 1:2]
            # me = glob/N + [0, EPS]  ->  [mean, ex2+EPS]
            nc.vector.scalar_tensor_tensor(
                me[:], glob[:], 1.0 / N_GLOB, epscol[:], ALU.mult, ALU.add
            )
            nc.vector.tensor_tensor(msq, mean, mean, ALU.mult)
            nc.vector.tensor_tensor(veps, e2e, msq, ALU.subtract)
            # rsqrt(veps): ACT sqrt + DVE reciprocal (optional Newton step on
            # the sqrt; scl folds in its 1/2)
            nc.scalar.activation(s0, veps, AF.Sqrt)
            nc.vector.reciprocal(r0, s0)
            if NEWTON:
                nc.vector.scalar_tensor_tensor(tnw, veps, r0, s0, ALU.mult, ALU.add)
                nc.vector.reciprocal(inv, tnw)
                nc.vector.tensor_scalar(scl, inv, g_sb[:], 2.0, ALU.mult, ALU.mult)
            else:
                nc.vector.tensor_scalar(scl, r0, g_sb[:], None, ALU.mult)
            # bia = beta - mean*scl
            nc.vector.tensor_scalar(bia, mean, scl, -1.0, ALU.mult, ALU.mult)
            nc.vector.tensor_tensor(bia, bia, b_sb[:], ALU.add)
            return scl, bia

        # ============ conv1 + BN1 stats ============
        conv(
            xp_sb,
            w1_sb,
            stat_writer(
                lambda b, t: y1p[b][:, 1 + t * ROWS : 1 + (t + 1) * ROWS, 1 : 1 + W],
                st1,
            ),
        )
        glob1 = sync_stats(st1, "1")
        scl1, bia1 = bn_coef(glob1, bn_par["g1"], bn_par["b1"], "1")

        # normalize + relu, in place (interior only; border stays zero).
        # image 0 is split to track conv2's small leading groups.
        norm_chunks = [
            (0, 0, 10), (0, 10, 8), (0, 18, 16), (0, 34, 22),
            (1, 0, 56), (2, 0, 56), (3, 0, 56),
        ]
        for b, r0, nr in norm_chunks:
            itr = y1p[b][:, 1 + r0 : 1 + r0 + nr, 1 : 1 + W]
            nc.scalar.activation(itr, itr, AF.Relu, bias=bia1, scale=scl1)

        # ============ conv2 + BN2 stats ============
        conv(
            y1p,
            w2_sb,
            stat_writer(
                lambda b, t: y2[b][:, t * ROWS : (t + 1) * ROWS, :],
                st2,
            ),
        )
        glob2 = sync_stats(st2, "2")
        scl2, bia2 = bn_coef(glob2, bn_par["g2"], bn_par["b2"], "2")

        # ============ final: relu(y2*scl2 + bia2 + x) ============
        # scl2*y2 + x is accumulated on the (idle) TensorEngine: a diag(scl2)
        # matmul plus an identity matmul per 8-row tile. The relu(+bia2)
        # drains split between ACT and DVE (alternating tiles) writing bf16
        # in place over y2; output DMAs go in 28-row chunks on two queues.
        nc.vector.tensor_scalar(diag_bf[:], id128[:], scl2, None, ALU.mult)
        for b in range(B_SH):
            for t in range(TPB):
                r0 = t * ROWS
                ys = y2[b][:, r0 : r0 + ROWS, :]
                xs = xp_sb[b][:, 1 + r0 : 1 + r0 + ROWS, 1 : 1 + W]
                ps = pp.tile([C, ROWS, W], F32, name="ps", tag="ps")
                nc.tensor.matmul(ps[:], diag_bf[:], ys, start=True, stop=False)
                nc.tensor.matmul(ps[:], id_bf[:], xs, start=False, stop=True)
                if t % 2 == 0:
                    nc.scalar.activation(ys, ps[:], AF.Relu, bias=bia2, scale=1.0)
                else:
                    nc.vector.tensor_scalar(
                        ys, ps[:], bia2, 0.0, ALU.add, ALU.max
                    )
                if t % 4 == 3 or t == TPB - 1:
                    # rows [lo, hi) of this image are final -> ship them
                    lo = (t // 4) * 4 * ROWS
                    hi = (t + 1) * ROWS
                    eng = nc.sync if (t // 4) % 2 == 0 else nc.gpsimd
                    eng.dma_start(
                        out_d[b][:, lo:hi, :], y2[b][:, lo:hi, :]
                    )

    _dedupe_ldweights(nc)
    return nc


@functools.lru_cache(maxsize=1)
def get_nc():
    nc = _build()
    nc.compile()
    return nc


def make_in_maps(x, w1, gamma1, beta1, w2, gamma2, beta2):
    x = np.ascontiguousarray(np.asarray(x, dtype=np.float32))
    xp = np.zeros((B, C, HP, WP), ml_dtypes.bfloat16)
    xp[:, :, 1 : 1 + H, 1 : 1 + W] = x.astype(ml_dtypes.bfloat16)
    # w[o,i,ky,kx] -> [i, (ky,kx,o)] so tap t's lhsT slice is [C_in, C_out]
    w1t = np.ascontiguousarray(
        np.asarray(w1, np.float32).transpose(1, 2, 3, 0)
    ).reshape(C, 9 * C).astype(ml_dtypes.bfloat16)
    w2t = np.ascontiguousarray(
        np.asarray(w2, np.float32).transpose(1, 2, 3, 0)
    ).reshape(C, 9 * C).astype(ml_dtypes.bfloat16)
    g1 = np.ascontiguousarray(np.asarray(gamma1, np.float32).reshape(C, 1))
    b1 = np.ascontiguousarray(np.asarray(beta1, np.float32).reshape(C, 1))
    g2 = np.ascontiguousarray(np.asarray(gamma2, np.float32).reshape(C, 1))
    b2 = np.ascontiguousarray(np.asarray(beta2, np.float32).reshape(C, 1))
    maps = []
    for i in range(N_CORES):
        maps.append(
            {
                "xp": np.ascontiguousarray(xp[i * B_SH : (i + 1) * B_SH]),
                "w1t": w1t,
                "w2t": w2t,
                "g1": g1,
                "b1": b1,
                "g2": g2,
                "b2": b2,
            }
        )
    return maps


def run(in_maps, trace=False, **kwargs):
    nc = get_nc()
    return run_bass_kernel_spmd(
        nc, in_maps, core_ids=list(range(N_CORES)), trace=trace, **kwargs
    )


def kernel(x, w1, gamma1, beta1, w2, gamma2, beta2):
    maps = make_in_maps(x, w1, gamma1, beta1, w2, gamma2, beta2)
    res = run(maps)
    out = np.concatenate([res.results[i]["out"] for i in range(N_CORES)], axis=0)
    return np.ascontiguousarray(out.astype(np.float32))
